# revision 14
# baseline (speedup 1.0000x reference)
"""Trainium2 Bass kernel for nn_APPAP (ASPP + positional attention), 8 NeuronCores.

Sharding: data-parallel over batch B=4 x row-halves (2 cores per sample).
Core (b, h) convolves rows [32h, 32h+32) of sample b (halo via host padding),
computes q/k/v_T for its half, AllGathers k and v_T within the sample pair,
then computes full softmax attention for its 2048 query pixels against all
4096 key pixels and writes gamma*out + x for its half.
"""

import os
import sys

import numpy as np

try:
    import concourse.bass as bass
except ImportError:  # container fallback path
    sys.path.insert(0, "/opt/trn_rl_repo")
    import concourse.bass as bass

import concourse.bacc as bacc
import concourse.mybir as mybir
import concourse.tile as tile
from concourse.bass_utils import run_bass_kernel_spmd
from contextlib import ExitStack

F32 = mybir.dt.float32
MM_DT = mybir.dt.float32r  # matmul compute dtype (bitcast view of f32 tiles)

B, C, H, W = 4, 512, 64, 64
HALF = 32                       # rows per core
NH = HALF * W                   # 2048 query pixels per core
N = H * W                       # 4096 key pixels per sample
PAD = 6                         # max halo (dilation 6)
HP, WP = HALF + 2 * PAD, W + 2 * PAD   # 44 x 76 padded window
CI_T = C // 128                 # 4 channel tiles
NT = NH // 512                  # 4 output pixel tiles per core
EPS = 1e-5

_CACHE = {}


def _mm(ap):
    return ap


def build():
    nc = bacc.Bacc("TRN2", target_bir_lowering=False, debug=False, num_devices=8)
    dt = F32
    rt = MM_DT

    # ---------------- DRAM parameters ----------------
    xpad = nc.declare_dram_parameter("xpad", [C, HP, WP], rt, isOutput=False)
    xhalf = nc.declare_dram_parameter("xhalf", [C, NH], rt, isOutput=False)
    w1T = nc.declare_dram_parameter("w1T", [C, 128], rt, isOutput=False)
    w2T = nc.declare_dram_parameter("w2T", [9, C, 128], rt, isOutput=False)
    w3T = nc.declare_dram_parameter("w3T", [9, C, 128], rt, isOutput=False)
    w4T = nc.declare_dram_parameter("w4T", [9, C, 128], rt, isOutput=False)
    w5T = nc.declare_dram_parameter("w5T", [C, 128], rt, isOutput=False)
    wqT = nc.declare_dram_parameter("wqT", [640, 128], rt, isOutput=False)
    wkT = nc.declare_dram_parameter("wkT", [640, 128], rt, isOutput=False)
    wvT = nc.declare_dram_parameter("wvT", [C, C], rt, isOutput=False)
    invp = nc.declare_dram_parameter("invp", [128, 5], dt, isOutput=False)  # col4 /4096
    bnbp = nc.declare_dram_parameter("bnbp", [128, 5], dt, isOutput=False)
    bq = nc.declare_dram_parameter("bq", [128, 1], dt, isOutput=False)
    bk = nc.declare_dram_parameter("bk", [128, 1], dt, isOutput=False)
    bv = nc.declare_dram_parameter("bv", [1, C], rt, isOutput=False)
    gam = nc.declare_dram_parameter("gam", [1, 1], dt, isOutput=False)
    idn = nc.declare_dram_parameter("idn", [128, 128], rt, isOutput=False)
    onr = nc.declare_dram_parameter("onr", [1, 128], rt, isOutput=False)
    onc = nc.declare_dram_parameter("onc", [128, 1], rt, isOutput=False)
    out = nc.declare_dram_parameter("out", [C, NH], dt, isOutput=True)

    # collective bounce buffers (internal DRAM)
    xsum_in = nc.dram_tensor("xsum_in", [128, CI_T, 2], rt)
    xsum_out = nc.dram_tensor("xsum_out", [128, CI_T, 2], rt)
    k_in = nc.dram_tensor("k_in", [128, NH], rt)
    k_out = nc.dram_tensor("k_out", [256, NH], rt)
    v_in = nc.dram_tensor("v_in", [16, 128, C], rt)
    v_out = nc.dram_tensor("v_out", [32, 128, C], rt)

    PAIRS = [[0, 1], [2, 3], [4, 5], [6, 7]]
    AF = mybir.ActivationFunctionType
    ALU = mybir.AluOpType

    with tile.TileContext(nc) as tc, ExitStack() as top:
        persist = top.enter_context(tc.tile_pool(name="persist", bufs=1))
        consts = top.enter_context(tc.tile_pool(name="consts", bufs=1))

        # ---------- constants / small vectors ----------
        ones_r = consts.tile([1, 128], rt)        # row of ones (K=1 rank-1 lhsT)
        ones_c = consts.tile([128, 1], rt)        # column of ones (Z accumulation)
        nc.sync.dma_start(ones_r[:], onr[:])
        nc.sync.dma_start(ones_c[:], onc[:])
        idn_sb = consts.tile([128, 128], rt)
        nc.sync.dma_start(idn_sb[:], idn[:])
        inv_sb = consts.tile([128, 5], dt)
        bnb_sb = consts.tile([128, 5], dt)
        nc.sync.dma_start(inv_sb[:], invp[:])
        nc.sync.dma_start(bnb_sb[:], bnbp[:])
        bq_sb = consts.tile([128, 1], dt)
        bk_sb = consts.tile([128, 1], dt)
        bv_sb = consts.tile([1, C], rt)
        gam_sb = consts.tile([1, 1], dt)
        nc.sync.dma_start(bq_sb[:], bq[:])
        nc.sync.dma_start(bk_sb[:], bk[:])
        nc.sync.dma_start(bv_sb[:], bv[:])
        nc.sync.dma_start(gam_sb[:], gam[:])

        # persistent across phases
        q_sb = persist.tile([128, NH], rt)
        kfull = persist.tile([128, 2, NH], rt)      # [ck, half, m_local]
        negM = persist.tile([128, 16], rt)          # -rowmax per query chunk
        negMrow = persist.tile([1, NH], rt)         # transposed, [1, n]
        xh = [persist.tile([128, NH], rt, tag=f"xh{_}", name=f"xh{_}")
              for _ in range(CI_T)]
        for t in range(CI_T):
            nc.sync.dma_start(xh[t][:], xhalf[128 * t:128 * (t + 1), :])

        with ExitStack() as phase1:
            wbig_pool = phase1.enter_context(tc.tile_pool(name="wbig", bufs=1))
            wrot_pool = phase1.enter_context(tc.tile_pool(name="wrot", bufs=8))
            feat_pool = phase1.enter_context(tc.tile_pool(name="feat", bufs=1))
            cps = phase1.enter_context(tc.tile_pool(name="cpsum", bufs=1, space="PSUM"))
            sps = phase1.enter_context(tc.tile_pool(name="spsum", bufs=2, space="PSUM"))
            small = phase1.enter_context(tc.tile_pool(name="small", bufs=1))

            convscope = phase1.enter_context(ExitStack())
            xp_pool = convscope.enter_context(tc.tile_pool(name="xpad", bufs=1))
            xp = [xp_pool.tile([128, HP, WP], rt, tag=f"xp{_}", name=f"xp{_}")
                  for _ in range(CI_T)]
            for t in range(CI_T):
                nc.sync.dma_start(xp[t][:], xpad[128 * t:128 * (t + 1), :, :])

            # ---------- global-avg-pool partial sums + AllReduce ----------
            xs = small.tile([128, CI_T, 2], rt)
            with nc.allow_low_precision(reason="global-mean partial sums, f32r ok"):
                for t in range(CI_T):
                    for c2 in range(2):   # fp32r matmul needs free dim >= 2
                        nc.vector.reduce_sum(
                            xs[:, t, c2:c2 + 1],
                            xp[t][:, PAD:PAD + HALF, PAD:PAD + W],
                            axis=mybir.AxisListType.XY)
            nc.sync.dma_start(xsum_in[:], xs[:])
            nc.gpsimd.collective_compute(
                "AllReduce", ALU.add, replica_groups=PAIRS,
                ins=[xsum_in[:].opt()], outs=[xsum_out[:].opt()])
            xsr = small.tile([128, CI_T, 2], rt)
            nc.sync.dma_start(xsr[:], xsum_out[:])

            # ---------- small weights (resident) ----------
            w5 = [wbig_pool.tile([128, 128], rt, tag=f"w5_{_}", name=f"w5_{_}")
                  for _ in range(CI_T)]
            for t in range(CI_T):
                nc.sync.dma_start(w5[t][:], w5T[128 * t:128 * (t + 1), :])
            wq = [wbig_pool.tile([128, 128], rt, tag=f"wq{_}", name=f"wq{_}")
                  for _ in range(5)]
            wk = [wbig_pool.tile([128, 128], rt, tag=f"wk{_}", name=f"wk{_}")
                  for _ in range(5)]
            for b_ in range(5):
                nc.sync.dma_start(wq[b_][:], wqT[128 * b_:128 * (b_ + 1), :])
                nc.sync.dma_start(wk[b_][:], wkT[128 * b_:128 * (b_ + 1), :])
            wv = [wbig_pool.tile([128, C], rt, tag=f"wv{_}", name=f"wv{_}")
                  for _ in range(CI_T)]
            for t in range(CI_T):
                nc.sync.dma_start(wv[t][:], wvT[128 * t:128 * (t + 1), :])

            # ---------- ASPP conv branches (own half) ----------
            # Loop order: branch -> tap -> ci (weight streamed, used for all
            # 4 output tiles back-to-back) -> output tile t.
            DIL = {1: 2, 2: 3, 3: 6}
            feat = [feat_pool.tile([128, NH], rt, tag=f"feat{b_}", name=f"feat{b_}")
                    for b_ in range(4)]
            wsrcs = {1: w2T, 2: w3T, 3: w4T}
            for br in range(4):
                ps = [cps.tile([128, 512], dt, tag=f"convps{_}", name=f"ps{br}_{_}")
                      for _ in range(NT)]
                taps = [(1, 1)] if br == 0 else [(k // 3, k % 3) for k in range(9)]
                d = 0 if br == 0 else DIL[br]
                nmm = len(taps) * CI_T
                imm = 0
                for (ky, kx) in taps:
                    for ci in range(CI_T):
                        wt_ = wrot_pool.tile([128, 128], rt, tag="wd",
                                             name=f"wd{br}_{ky}{kx}_{ci}")
                        if br == 0:
                            nc.sync.dma_start(wt_[:], w1T[128 * ci:128 * (ci + 1), :])
                        else:
                            nc.sync.dma_start(
                                wt_[:],
                                wsrcs[br][3 * ky + kx, 128 * ci:128 * (ci + 1), :])
                        for t in range(NT):
                            ro = 8 * t + PAD + (ky - 1) * d
                            co = PAD + (kx - 1) * d
                            nc.tensor.matmul(
                                ps[t][:].rearrange("p (a b) -> p a b", a=8),
                                _mm(wt_[:]),
                                _mm(xp[ci][:, ro:ro + 8, co:co + W]),
                                start=(imm == 0), stop=(imm == nmm - 1))
                        imm += 1
                for t in range(NT):
                    nc.scalar.activation(
                        feat[br][:, 512 * t:512 * (t + 1)], ps[t][:], AF.Relu,
                        bias=bnb_sb[:, br:br + 1], scale=inv_sb[:, br:br + 1])

            convscope.close()   # frees xpad tiles

            # ---------- branch 5 (global pool) + q/k biases ----------
            ps5 = sps.tile([128, 2], dt, tag="sp")
            for ci in range(CI_T):
                nc.tensor.matmul(ps5[:], _mm(w5[ci][:]), _mm(xsr[:, ci, :]),
                                 start=(ci == 0), stop=(ci == CI_T - 1))
            b5 = small.tile([128, 2], rt)
            nc.scalar.activation(b5[:], ps5[:], AF.Relu,
                                 bias=bnb_sb[:, 4:5], scale=inv_sb[:, 4:5])
            psq5 = sps.tile([128, 2], dt, tag="sp")
            nc.tensor.matmul(psq5[:], _mm(wq[4][:]), _mm(b5[:]), start=True, stop=True)
            qbias = small.tile([128, 1], dt)
            nc.vector.tensor_add(qbias[:], psq5[:, 0:1], bq_sb[:])
            psk5 = sps.tile([128, 2], dt, tag="sp")
            nc.tensor.matmul(psk5[:], _mm(wk[4][:]), _mm(b5[:]), start=True, stop=True)
            kbias = small.tile([128, 1], dt)
            nc.vector.tensor_add(kbias[:], psk5[:, 0:1], bk_sb[:])

            # ---------- q and k (own half) ----------
            k_own = small.tile([128, NH], rt)
            for t in range(NT):
                sl = slice(512 * t, 512 * (t + 1))
                psq = sps.tile([128, 512], dt, tag="sp")
                for br in range(4):
                    nc.tensor.matmul(psq[:], _mm(wq[br][:]), _mm(feat[br][:, sl]),
                                     start=(br == 0), stop=(br == 3))
                nc.scalar.activation(q_sb[:, sl], psq[:], AF.Identity, bias=qbias[:])
                psk = sps.tile([128, 512], dt, tag="sp")
                for br in range(4):
                    nc.tensor.matmul(psk[:], _mm(wk[br][:]), _mm(feat[br][:, sl]),
                                     start=(br == 0), stop=(br == 3))
                nc.scalar.activation(k_own[:, sl], psk[:], AF.Identity, bias=kbias[:])
            nc.sync.dma_start(k_in[:], k_own[:])
            nc.gpsimd.collective_compute(
                "AllGather", ALU.bypass, replica_groups=PAIRS,
                ins=[k_in[:].opt()], outs=[k_out[:].opt()])

            # ---------- v_T (own half): v_T[m, c] ----------
            vst_pool = phase1.enter_context(tc.tile_pool(name="vst", bufs=3))
            for mc in range(16):
                psv = sps.tile([128, C], dt, tag="sp")
                for ci in range(CI_T):
                    nc.tensor.matmul(
                        psv[:],
                        _mm(xh[ci][:, 128 * mc:128 * (mc + 1)]),
                        _mm(wv[ci][:]), start=(ci == 0), stop=False)
                nc.tensor.matmul(psv[:], _mm(ones_r[:]), _mm(bv_sb[:]),
                                 start=False, stop=True)
                vst = vst_pool.tile([128, C], rt, tag="vst")
                nc.scalar.copy(vst[:], psv[:])
                nc.sync.dma_start(v_in[mc], vst[:])
            nc.gpsimd.collective_compute(
                "AllGather", ALU.bypass, replica_groups=PAIRS,
                ins=[v_in[:].opt()], outs=[v_out[:].opt()])

            # k gather back to SBUF (persistent)
            nc.sync.dma_start(kfull[:], k_out[:].rearrange("(g p) m -> p g m", p=128))

        kf = kfull[:].rearrange("p g m -> p (g m)")

        # ---------- pass 1: row maxima of energy ----------
        with ExitStack() as phase2:
            eps_pool = phase2.enter_context(
                tc.tile_pool(name="epsum", bufs=3, space="PSUM"))
            rps_pool = phase2.enter_context(
                tc.tile_pool(name="rpsum", bufs=2, space="PSUM"))
            mx_pool = phase2.enter_context(tc.tile_pool(name="mx", bufs=2))

            for i in range(16):
                tmax = mx_pool.tile([128, 8], dt, tag="tmax")
                for mt in range(8):
                    pse = eps_pool.tile([128, 512], dt, tag="eps")
                    nc.tensor.matmul(pse[:], _mm(q_sb[:, 128 * i:128 * (i + 1)]),
                                     _mm(kf[:, 512 * mt:512 * (mt + 1)]),
                                     start=True, stop=True)
                    nc.vector.reduce_max(tmax[:, mt:mt + 1], pse[:],
                                         axis=mybir.AxisListType.X)
                nc.vector.tensor_reduce(negM[:, i:i + 1], tmax[:],
                                        axis=mybir.AxisListType.X,
                                        op=mybir.AluOpType.max, negate=True)
            # transpose negM -> negMrow [1, NH]
            for j in range(NT):
                psr = rps_pool.tile([1, 512], dt, tag="psr")
                for ii in range(4):
                    i = 4 * j + ii
                    nc.tensor.matmul(psr[:, 128 * ii:128 * (ii + 1)],
                                     _mm(negM[:, i:i + 1]), _mm(idn_sb[:]),
                                     start=True, stop=True)
                nc.scalar.copy(negMrow[:, 512 * j:512 * (j + 1)], psr[:])

        # ---------- pass 2: exp, Z, att @ v, epilogue ----------
        with ExitStack() as phase3:
            vf_pool = phase3.enter_context(tc.tile_pool(name="vf", bufs=1))
            eps2 = phase3.enter_context(
                tc.tile_pool(name="eps2", bufs=2, space="PSUM"))
            ops_pool = phase3.enter_context(
                tc.tile_pool(name="opsum", bufs=1, space="PSUM"))
            zps_pool = phase3.enter_context(
                tc.tile_pool(name="zpsum", bufs=1, space="PSUM"))
            bps_pool = phase3.enter_context(
                tc.tile_pool(name="bpsum", bufs=1, space="PSUM"))
            u_pool = phase3.enter_context(tc.tile_pool(name="u", bufs=3))
            f_pool = phase3.enter_context(tc.tile_pool(name="fin", bufs=3))

            vfull = vf_pool.tile([128, 32, C], rt)   # [m_in_chunk, chunk, c]
            nc.sync.dma_start(vfull[:], v_out[:].rearrange("g p c -> p g c"))

            for j in range(NT):
                nsl = slice(512 * j, 512 * (j + 1))
                ops = [ops_pool.tile([128, 512], dt, tag=f"ops{ct}", name=f"ops{ct}")
                       for ct in range(4)]
                zps = zps_pool.tile([1, 512], dt, tag="zps")
                for mc in range(32):
                    pse = eps2.tile([128, 512], dt, tag="e2")
                    nc.tensor.matmul(pse[:], _mm(kf[:, 128 * mc:128 * (mc + 1)]),
                                     _mm(q_sb[:, nsl]), start=True, stop=False)
                    nc.tensor.matmul(pse[:], _mm(ones_r[:]), _mm(negMrow[:, nsl]),
                                     start=False, stop=True)
                    u = u_pool.tile([128, 512], rt, tag="u")
                    nc.scalar.activation(u[:], pse[:], AF.Exp)
                    nc.tensor.matmul(zps[:], _mm(ones_c[:]), _mm(u[:]),
                                     start=(mc == 0), stop=(mc == 31))
                    for ct in range(4):
                        nc.tensor.matmul(ops[ct][:],
                                         _mm(vfull[:, mc, 128 * ct:128 * (ct + 1)]),
                                         _mm(u[:]), start=(mc == 0), stop=(mc == 31))
                # normalization row: gamma / Z
                srow = f_pool.tile([1, 512], rt, tag="srow")
                with nc.allow_low_precision(reason="1/Z normalization row, f32r ok"):
                    nc.vector.reciprocal(srow[:], zps[:])
                nc.vector.tensor_scalar_mul(srow[:], srow[:], gam_sb[:, 0:1])
                bps = bps_pool.tile([128, 512], dt, tag="bps")
                nc.tensor.matmul(bps[:], _mm(ones_r[:]), _mm(srow[:]),
                                 start=True, stop=True)
                bcs = f_pool.tile([128, 512], dt, tag="bcs")
                nc.scalar.copy(bcs[:], bps[:])
                for ct in range(4):
                    fin = f_pool.tile([128, 512], dt, tag="fin")
                    nc.vector.tensor_mul(fin[:], ops[ct][:], bcs[:])
                    nc.vector.tensor_add(fin[:], fin[:], xh[ct][:, nsl])
                    nc.sync.dma_start(out[128 * ct:128 * (ct + 1), nsl], fin[:])

    nc.compile()
    return nc


def _prep_shared(inputs):
    f = np.float32
    inv = (inputs["bn_scale"] / np.sqrt(1.0 + EPS)).astype(f)          # [5,128]
    invp = inv.T.copy()
    invp[:, 4] /= float(N)                                             # fold mean /N
    shared = {
        "w1T": np.ascontiguousarray(inputs["w_a1"].reshape(128, C).T).astype(f),
        "w2T": np.ascontiguousarray(
            inputs["w_a2"].transpose(2, 3, 1, 0).reshape(9, C, 128)).astype(f),
        "w3T": np.ascontiguousarray(
            inputs["w_a3"].transpose(2, 3, 1, 0).reshape(9, C, 128)).astype(f),
        "w4T": np.ascontiguousarray(
            inputs["w_a4"].transpose(2, 3, 1, 0).reshape(9, C, 128)).astype(f),
        "w5T": np.ascontiguousarray(inputs["w_a5"].reshape(128, C).T).astype(f),
        "wqT": np.ascontiguousarray(inputs["w_q"].reshape(128, 640).T).astype(f),
        "wkT": np.ascontiguousarray(inputs["w_k"].reshape(128, 640).T).astype(f),
        "wvT": np.ascontiguousarray(inputs["w_v"].reshape(C, C).T).astype(f),
        "invp": np.ascontiguousarray(invp),
        "bnbp": np.ascontiguousarray(inputs["bn_bias"].T).astype(f),
        "bq": inputs["b_q"].reshape(128, 1).astype(f),
        "bk": inputs["b_k"].reshape(128, 1).astype(f),
        "bv": inputs["b_v"].reshape(1, C).astype(f),
        "gam": inputs["gamma"].reshape(1, 1).astype(f),
        "idn": np.eye(128, dtype=f),
        "onr": np.ones((1, 128), dtype=f),
        "onc": np.ones((128, 1), dtype=f),
    }
    return shared


def kernel(**inputs):
    if "nc" not in _CACHE:
        _CACHE["nc"] = build()
    nc = _CACHE["nc"]

    x = np.asarray(inputs["x"], dtype=np.float32)
    shared = _prep_shared({k: np.asarray(v) for k, v in inputs.items()})

    in_maps = []
    for core in range(8):
        b, h = core // 2, core % 2
        xpad = np.zeros((C, HP, WP), dtype=np.float32)
        lo, hi = 32 * h - PAD, 32 * h + HALF + PAD
        slo, shi = max(lo, 0), min(hi, H)
        xpad[:, slo - lo:shi - lo, PAD:PAD + W] = x[b, :, slo:shi, :]
        xhalf = np.ascontiguousarray(
            x[b, :, 32 * h:32 * h + HALF, :].reshape(C, NH))
        m = dict(shared)
        m["xpad"] = xpad
        m["xhalf"] = xhalf
        in_maps.append(m)

    trace = bool(os.environ.get("KERNEL_TRACE"))
    res = run_bass_kernel_spmd(nc, in_maps, core_ids=list(range(8)), trace=trace)
    if trace:
        _CACHE["exec_time_ns"] = res.exec_time_ns

    full = np.empty((B, C, H, W), dtype=np.float32)
    for core in range(8):
        b, h = core // 2, core % 2
        full[b, :, 32 * h:32 * h + HALF, :] = \
            res.results[core]["out"].reshape(C, HALF, W)
    return full
